# revision 1
# baseline (speedup 1.0000x reference)
"""Trainium2 Bass kernel for DensePairwiseRelaxedWordMoverSimilarity.

Shapes (hardcoded): x1 (64,128,512) f32, mask1 (64,128) bool,
                    x2 (64,128,512) f32, mask2 (64,128) bool -> out (64,64) f32.

Strategy: data-parallel over B1 across 8 cores; each core holds an 8-row
x1 slab plus the full x2 and computes an (8, 64) output slab.

v2: fp8 datapath. Host normalizes both sides (cosine sim = dot of unit
vectors), zeroes masked rows, pre-scales by 32 and quantizes to e4m3.
Device runs the (B1/8*S1) x D x (B2*S2) pairwise dot as fp8 DoubleRow
matmuls (2 k-tiles per instruction, 0.5 cycles/row -> 4x the f32r rate),
copies each PSUM slab to bf16 (undoing the 32*32 pre-scale), takes
segmented reduce_max for sim2, PE-transposes the bf16 slab (1.0
cycles/row) into bf16 PSUM and reduce_maxes for sim1, then computes the
masked means with tiny bf16 matmuls. Collections are laid out [p, B2, A]
so reduce outputs are stride-1 (DVE 2x_1p eligible). Sequence-length
sorting/trimming and the length-sorted round-robin deal of x1 rows to
cores is as in the f32r version (uniform trims are SPMD-safe); even-count
rounding keeps bf16 segment starts 4B-aligned.

Transposes of slab b-1 are emitted between the matmuls of slab b so the
PE never stalls on the ACT copy; masked rows are zeroed so they lose
every max (valid sims are never all-negative for this data).
"""

import numpy as np
import ml_dtypes

import concourse.bacc as bacc
import concourse.mybir as mybir
from concourse import bass_isa
from concourse import tile
from concourse.bass_utils import run_bass_kernel_spmd

F32 = mybir.dt.float32
BF16 = mybir.dt.bfloat16
FP8 = mybir.dt.float8e4
AX = mybir.AxisListType
AF = mybir.ActivationFunctionType
ALU = mybir.AluOpType
DR = mybir.MatmulPerfMode.DoubleRow

E4NP = ml_dtypes.float8_e4m3
BFNP = ml_dtypes.bfloat16

B1, S1, B2, S2, D = 64, 128, 64, 128, 512
NCORES = 8
A = B1 // NCORES          # 8 a-slots per core
X1W = A * S1              # 1024 x1 columns
X2W = B2 * S2             # 8192 x2 columns
NG2 = X2W // 512          # 16 x2 column groups (4 b each)
QSCALE = 32.0             # fp8 pre-scale; sims carry QSCALE^2
GB0, GB_N = 16, 32        # b-slabs [GB0, GB0+GB_N) take the gpsimd sim1 path

_CACHE = {}


def _patch_compile_flags():
    """Enable walrus ldweights dedup (consecutive identical stationary
    operands skip the reload)."""
    import concourse.bass_utils as bu

    if getattr(bu, "_ldw_opt_patched", False):
        return
    orig = bu.run_command

    def patched(cmd, **kw):
        cmd = [
            c.replace("--enable-ldw-opt=false", "--enable-ldw-opt=true")
            if isinstance(c, str)
            else c
            for c in cmd
        ]
        return orig(cmd, **kw)

    bu.run_command = patched
    bu._ldw_opt_patched = True


def _build(L1h, L2):
    """L1h: baked s-trims for the two a-slot halves (uniform across cores).
    L2: 64 baked t-trims (per sorted b position).

    NOTE: walrus's ldw-opt pass crashes on bf16 transpose ldweights, so
    unlike the f32r version we must NOT enable --enable-ldw-opt."""
    nc = bacc.Bacc(None, target_bir_lowering=False, debug=False)

    x1p = nc.declare_dram_parameter("x1p", [D, X1W], FP8, isOutput=False)
    x2p = nc.declare_dram_parameter("x2p", [D, X2W], FP8, isOutput=False)
    identp = nc.declare_dram_parameter("identp", [128, 128], BF16, isOutput=False)
    consts = nc.declare_dram_parameter("consts", [128, 73], BF16, isOutput=False)
    w1rowp = nc.declare_dram_parameter("w1rowp", [1, A * S1], F32, isOutput=False)
    out = nc.declare_dram_parameter("out", [1, A * B2], F32, isOutput=True)
    out2 = nc.declare_dram_parameter("out2", [GB_N, A], F32, isOutput=True)

    inv_q = float(1.0 / (QSCALE * QSCALE))

    with tile.TileContext(nc) as tc:
        with (
            tc.tile_pool(name="xts", bufs=1) as xts_pool,
            tc.tile_pool(name="cst", bufs=1) as cst_pool,
            tc.tile_pool(name="coll", bufs=1) as coll_pool,
            tc.tile_pool(name="cpool", bufs=4) as cpool,
            tc.tile_pool(name="gred", bufs=2) as gred_pool,
            tc.tile_pool(name="mtmp", bufs=2) as mtmp_pool,
            tc.tile_pool(name="psS", bufs=4, space="PSUM") as psS,
            tc.tile_pool(name="psT", bufs=3, space="PSUM") as psT,
            tc.tile_pool(name="psm", bufs=1, space="PSUM") as psm,
        ):
            # ---- loads: constants first (FIFO rings), then x1, then x2 in
            # graduated blocks so the first matmul can start ~3us in.
            ident = cst_pool.tile([128, 128], BF16, tag="ident")
            nc.sync.dma_start(ident[:], identp.ap())
            csts = cst_pool.tile([128, 73], BF16, tag="consts")
            nc.sync.dma_start(csts[:], consts.ap())
            x1t = xts_pool.tile([128, 4, X1W], FP8, tag="x1t")
            nc.sync.dma_start(
                x1t[:], x1p.ap().rearrange("(k p) m -> p k m", p=128)
            )
            x2g = []
            blocks = [(0, 1), (1, 1), (2, 2), (4, 4), (8, 8)]
            for g0, w in blocks:
                xb = xts_pool.tile([128, 4, w * 512], FP8, tag=f"xb{g0}")
                nc.gpsimd.dma_start(
                    xb[:],
                    x2p.ap()[:, g0 * 512 : (g0 + w) * 512].rearrange(
                        "(k p) m -> p k m", p=128
                    ),
                )
                for j in range(w):
                    x2g.append(xb[:, :, j * 512 : (j + 1) * 512])

            w1T = csts[:, 0:8]
            w2T = csts[:, 8:72]
            onescol = csts[:, 72:73]

            # collections, [p, b, a] so reduce outputs are stride-1
            sim1st = coll_pool.tile([128, B2, A], BF16, tag="sim1st")
            sim2st = coll_pool.tile([128, B2, A], BF16, tag="sim2st")
            # gpsimd-path sim1 rows, [offloaded b, (a s)]; their sim1st
            # slots stay zero (memset) so the PE means see a 0 m1-part and
            # the host adds out2 back in.
            sim1g = coll_pool.tile([GB_N, A * S1], F32, tag="sim1g")
            w1bc = coll_pool.tile([GB_N, A * S1], F32, tag="w1bc")
            nc.sync.dma_start(w1bc[:], w1rowp.ap().to_broadcast((GB_N, A * S1)))
            nc.vector.memset(sim1st[:], 0.0)

            def emit_mm(b):
                """fp8 DoubleRow matmuls + bf16 copy for slab b."""
                g = b // 4
                boff = (b % 4) * 128
                l2 = L2[b]
                Cs = []
                for h in range(2):
                    S = psS.tile([128, 512], F32, tag="S", name=f"S{b}_{h}")
                    # operands are untrimmed (zero-padded by the host; zeros
                    # never win a max and DoubleRow cost depends only on the
                    # moving width) so PSUM/SBUF tiles are always fully
                    # written — trims apply only to transpose widths and
                    # reduce extents. h=1 walks pairs in reverse so boundary
                    # matmuls share a stationary operand (ldw-opt skips the
                    # reload).
                    prs = ((0, 2), (2, 4)) if h == 0 else ((2, 4), (0, 2))
                    for i, (k0, k1) in enumerate(prs):
                        nc.tensor.matmul(
                            S[:],
                            x2g[g][:, k0:k1, boff : boff + 128],
                            x1t[:, k0:k1, 512 * h : 512 * h + 512],
                            start=(i == 0),
                            stop=(i == 1),
                            perf_mode=DR,
                        )
                    C = cpool.tile([128, 512], BF16, tag="C", name=f"C{b}_{h}")
                    nc.scalar.activation(C[:], S[:], AF.Copy, scale=inv_q)
                    Cs.append(C)
                return Cs

            def emit_red(b, Cs):
                """sim2 reduce always on DVE; sim1 via PE transpose + DVE
                reduce, except for GB_N slabs whose sim1 is a gpsimd
                partition-max (axis=C) straight off C — DVE is the
                bottleneck engine and gpsimd idles after the input DMAs."""
                l2 = L2[b]
                gpath = GB0 <= b < GB0 + GB_N
                for h in range(2):
                    l1 = L1h[h]
                    C = Cs[h]
                    nc.vector.reduce_max(
                        sim2st[:, b, 4 * h : 4 * h + 4],
                        C[:].rearrange("p (a s) -> p a s", a=4)[:, :, 0:l1],
                        axis=AX.X,
                    )
                    if gpath:
                        gr = gred_pool.tile(
                            [128, 512], F32, tag="gr", name=f"gr{b}_{h}"
                        )
                        nc.gpsimd.partition_all_reduce(
                            gr[:], C[:], 128, bass_isa.ReduceOp.max
                        )
                        nc.sync.dma_start(
                            sim1g[b - GB0 : b - GB0 + 1, 512 * h : 512 * h + 512],
                            gr[0:1, :],
                        )
                        continue
                    T = psT.tile([128, 512], BF16, tag="T", name=f"T{b}_{h}")
                    for q in range(4):
                        nc.tensor.transpose(
                            T[:, q * l2 : (q + 1) * l2],
                            C[:, q * 128 : (q + 1) * 128],
                            ident[:, 0:l2],
                        )
                    nc.vector.reduce_max(
                        sim1st[:, b, 4 * h : 4 * h + 4],
                        T[:, 0 : 4 * l2].rearrange("p (a t) -> p a t", a=4),
                        axis=AX.X,
                    )

            prev = None
            for b in range(B2):
                Cs = emit_mm(b)
                if prev is not None:
                    emit_red(prev[0], prev[1])
                prev = (b, Cs)
            emit_red(prev[0], prev[1])

            # ---- means ----
            mps = psm.tile([1, A * B2], F32, tag="mps")
            for a in range(A):
                s2w = mtmp_pool.tile([128, B2], BF16, tag="s2w")
                nc.vector.tensor_mul(s2w[:], sim2st[:, :, a], w2T)
                nc.tensor.matmul(
                    mps[:, a * B2 : (a + 1) * B2],
                    w1T[:, a : a + 1],
                    sim1st[:, :, a],
                    start=True,
                    stop=False,
                )
                nc.tensor.matmul(
                    mps[:, a * B2 : (a + 1) * B2],
                    onescol,
                    s2w[:],
                    start=False,
                    stop=True,
                )
            outs = cst_pool.tile([1, A * B2], F32, tag="outs")
            nc.scalar.copy(outs[:], mps[:])
            nc.sync.dma_start(out.ap(), outs[:])

            # gpsimd-path means: one weighted-sum pass over the collected
            # sim1 rows (rows = offloaded slabs, cols = (a, s))
            s1w = coll_pool.tile([GB_N, A * S1], F32, tag="s1w")
            nc.vector.tensor_mul(s1w[:], sim1g[:], w1bc[:])
            m1g = coll_pool.tile([GB_N, A], F32, tag="m1g")
            nc.vector.tensor_reduce(
                m1g[:],
                s1w[:].rearrange("p (a s) -> p a s", a=A),
                axis=AX.X,
                op=ALU.add,
            )
            nc.sync.dma_start(out2.ap(), m1g[:])
    nc.finalize()
    return nc


def _prep(x1, mask1, x2, mask2):
    """Host-side marshaling: normalize, mask-zero, sort, quantize, deal."""
    x1 = np.asarray(x1, dtype=np.float32)
    x2 = np.asarray(x2, dtype=np.float32)
    m1 = np.asarray(mask1).astype(bool)
    m2 = np.asarray(mask2).astype(bool)

    EPS = 1e-8
    n1 = np.sqrt((x1 * x1).sum(-1, keepdims=True))
    n2 = np.sqrt((x2 * x2).sum(-1, keepdims=True))
    x1n = (x1 / np.maximum(n1, EPS)) * QSCALE
    x2n = (x2 / np.maximum(n2, EPS)) * QSCALE
    x1n[~m1] = 0.0
    x2n[~m2] = 0.0

    len1 = m1.sum(axis=1).astype(np.int64)
    len2 = m2.sum(axis=1).astype(np.int64)
    ext1 = np.where(m1.any(1), S1 - np.argmax(m1[:, ::-1], axis=1), 1)
    ext2 = np.where(m2.any(1), S2 - np.argmax(m2[:, ::-1], axis=1), 1)
    a_rank = np.argsort(-ext1, kind="stable")
    b_order = np.argsort(-ext2, kind="stable")
    a_slot = a_rank.reshape(A, NCORES)              # [slot, core]

    def _ev(v):
        v = int(max(v, 1))
        return v + (v % 2)   # even counts keep bf16 segments 4B-aligned

    L1slot = [_ev(ext1[a_slot[s]].max()) for s in range(A)]
    L1h = (max(L1slot[0:4]), max(L1slot[4:8]))
    L2 = tuple(_ev(ext2[b]) for b in b_order)

    l1f = np.maximum(len1, 1).astype(np.float32)
    l2f = np.maximum(len2, 1).astype(np.float32)
    w1 = m1.astype(np.float32) * (0.5 / l1f)[:, None]
    w2 = m2.astype(np.float32) * (0.5 / l2f)[:, None]

    x2s = x2n[b_order]
    w2T = np.ascontiguousarray(w2[b_order].T)        # [128 t, 64 bpos]
    x2T = np.ascontiguousarray(x2s.reshape(X2W, D).T).astype(E4NP)
    ident = np.eye(128, dtype=BFNP)

    in_maps = []
    for c in range(NCORES):
        aidx = a_slot[:, c]
        x1T = np.ascontiguousarray(
            x1n[aidx].reshape(X1W, D).T
        ).astype(E4NP)
        w1Tc = np.ascontiguousarray(w1[aidx].T)      # [128 s, 8 slot]
        constsc = np.concatenate(
            [w1Tc, w2T, np.ones((128, 1), np.float32)], axis=1
        ).astype(BFNP)
        w1row = np.ascontiguousarray(w1[aidx].reshape(1, A * S1))
        in_maps.append(
            {
                "x1p": x1T,
                "x2p": x2T,
                "identp": ident,
                "consts": np.ascontiguousarray(constsc),
                "w1rowp": w1row,
            }
        )
    return in_maps, a_slot, b_order, (L1h, L2)


def kernel(x1, mask1, x2, mask2):
    in_maps, a_slot, b_order, key = _prep(x1, mask1, x2, mask2)
    if _CACHE.get("key") != key:
        _CACHE["nc"] = _build(*key)
        _CACHE["key"] = key
    nc = _CACHE["nc"]
    res = run_bass_kernel_spmd(nc, in_maps, list(range(NCORES)))
    outp = np.zeros((B1, B2), dtype=np.float32)
    for c in range(NCORES):
        slab = res.results[c]["out"].reshape(A, B2).copy()  # [slot, sorted b]
        m1g = res.results[c]["out2"]                  # [offloaded b, slot]
        slab[:, GB0 : GB0 + GB_N] += m1g.T
        for s in range(A):
            outp[a_slot[s, c], b_order] = slab[s]
    return np.ascontiguousarray(outp)



# revision 19
# speedup vs baseline: 1.7787x; 1.7787x over previous
"""Trainium2 Bass kernel for DensePairwiseRelaxedWordMoverSimilarity.

Shapes (hardcoded): x1 (64,128,512) f32, mask1 (64,128) bool,
                    x2 (64,128,512) f32, mask2 (64,128) bool -> out (64,64) f32.

Strategy: data-parallel over B1 across 8 cores; core k holds x1 rows
[8k, 8k+8) plus the full x2 and produces an (8, 64) output slab.

v3: single-orientation fp8 matmul + exp-domain reductions.
For each (a, 4-b chunk) the PE computes C^T tiles [128 s, 4b x 128 t]
(fp8 DoubleRow, f32 PSUM).  The scalar engine applies
E = exp(beta*(C - c0)) into bf16 SBUF (trimmed to the chunk's baked
t-extent).  Then BOTH reductions come cheap:
  - sim1[a,b,s] = max_t C = (ln max_t E)/beta + c0: segmented DVE
    reduce_max over the free dim (optionally gpsimd pre-halving),
  - sim2[a,b,t] via log-sum-exp: sum_s E is a PE matmul with an
    all-ones stationary column, accumulated per (chunk, a) into one
    PSUM bank row; masked-s rows contribute exp(-beta*c0) ~ 1e-38 (b
    columns are sorted+trimmed so masked-t never pollutes sums).
    The LSE overshoot at beta=250 measures ~3e-3 rel on this data,
    under the fp8 quantization floor.
One Ln pass per collection (+1e-36 bias so memset-zero pad columns
stay finite) and tiny w1/w2-weighted matmul/reduce means finish on
device; host adds the (m1+m2)/beta + c0 affine and unsorts b.
"""

import numpy as np
import ml_dtypes

import concourse.bacc as bacc
import concourse.mybir as mybir
from concourse import tile
from concourse.bass_utils import run_bass_kernel_spmd

F32 = mybir.dt.float32
BF16 = mybir.dt.bfloat16
FP8 = mybir.dt.float8e4
AX = mybir.AxisListType
AF = mybir.ActivationFunctionType
ALU = mybir.AluOpType
DR = mybir.MatmulPerfMode.DoubleRow

E4NP = ml_dtypes.float8_e4m3
BFNP = ml_dtypes.bfloat16

B1, S1, B2, S2, D = 64, 128, 64, 128, 512
NCORES = 8
A = B1 // NCORES          # 8 x1 rows per core
X1W = A * S1              # 1024 x1 columns
X2W = B2 * S2             # 8192 x2 columns
NCH = 16                  # chunks of 4 sorted b's (512 x2 columns)
QSCALE = 32.0             # fp8 pre-scale; sims carry QSCALE^2
BETA = 250.0              # LSE sharpness
C0 = 0.35                 # exp recentering; keeps E <= ~1
LNEPS = 1e-36             # keeps ln() of pad columns finite
GPS_MOD = 0               # gpsimd pre-halving every GPS_MOD-th unit (0=off)

_CACHE = {}


def _build(L2G):
    """L2G: 16 baked t-extents (even), one per sorted 4-b chunk."""
    nc = bacc.Bacc(None, target_bir_lowering=False, debug=False)

    x1p = nc.declare_dram_parameter("x1p", [D, X1W], FP8, isOutput=False)
    x2p = nc.declare_dram_parameter("x2p", [D, X2W], FP8, isOutput=False)
    consts = nc.declare_dram_parameter("consts", [128, 4], F32, isOutput=False)
    w2patp = nc.declare_dram_parameter("w2patp", [128, 512], F32, isOutput=False)
    w1Tp = nc.declare_dram_parameter("w1Tp", [128, A], F32, isOutput=False)
    m1o = nc.declare_dram_parameter("m1o", [A, A * B2], F32, isOutput=True)
    m2o = nc.declare_dram_parameter("m2o", [128, 4], F32, isOutput=True)

    exp_scale = float(BETA / (QSCALE * QSCALE))

    with tile.TileContext(nc) as tc:
        with (
            tc.tile_pool(name="xts", bufs=1) as xts_pool,
            tc.tile_pool(name="cst", bufs=1) as cst_pool,
            tc.tile_pool(name="coll", bufs=1) as coll_pool,
            tc.tile_pool(name="epool", bufs=4) as epool,
            tc.tile_pool(name="hpool", bufs=3) as hpool,
            tc.tile_pool(name="psS", bufs=3, space="PSUM") as psS,
            tc.tile_pool(name="psF", bufs=1, space="PSUM") as psF,
            tc.tile_pool(name="psM", bufs=1, space="PSUM") as psM,
        ):
            # ---- loads: constants first, then x1, then x2 in graduated
            # chunk blocks so the first matmul can start early.
            csts = cst_pool.tile([128, 4], F32, tag="consts")
            nc.sync.dma_start(csts[:], consts.ap())
            w2pat = cst_pool.tile([128, 512], F32, tag="w2pat")
            nc.sync.dma_start(w2pat[:], w2patp.ap())
            w1T = cst_pool.tile([128, A], F32, tag="w1T")
            nc.sync.dma_start(w1T[:], w1Tp.ap())
            x1t = xts_pool.tile([128, 4, X1W], FP8, tag="x1t")
            nc.sync.dma_start(
                x1t[:], x1p.ap().rearrange("(k p) m -> p k m", p=128)
            )
            x2c = []
            blocks = [(0, 1), (1, 1), (2, 2), (4, 4), (8, 8)]
            for g0, w in blocks:
                xb = xts_pool.tile([128, 4, w * 512], FP8, tag=f"xb{g0}")
                nc.gpsimd.dma_start(
                    xb[:],
                    x2p.ap()[:, g0 * 512 : (g0 + w) * 512].rearrange(
                        "(k p) m -> p k m", p=128
                    ),
                )
                for j in range(w):
                    x2c.append(xb[:, :, j * 512 : (j + 1) * 512])

            expbias = csts[:, 1:2]      # -BETA*C0
            # sliding-window one-hot: col 64 is all-ones, so the width-64
            # slice [64-m : 128-m] has its ones in column m.
            oh64 = cst_pool.tile([128, 128], BF16, tag="oh64")
            nc.vector.memset(oh64[:], 0.0)
            nc.vector.tensor_copy(
                oh64[:, 64:65], csts[:, 0:1]
            )

            # maxE collection [s, a, sorted-b]; SumE PSUM bank rows (8c+a)
            sim1st = coll_pool.tile([128, A, B2], BF16, tag="sim1st")
            sumE = psF.tile([128, 512], F32, tag="sumE")
            nc.vector.memset(sumE[:], 0.0)

            def emit_mm(u):
                """fp8 DoubleRow matmuls for unit u = (chunk, a-pair)."""
                c, ap_ = u // 4, (u % 4) * 2
                S = psS.tile([128, 2, 512], F32, tag="S", name=f"S{u}")
                for j in range(2):
                    a = ap_ + j
                    for i, (k0, k1) in enumerate(((0, 2), (2, 4))):
                        nc.tensor.matmul(
                            S[:, j, :],
                            x1t[:, k0:k1, a * 128 : (a + 1) * 128],
                            x2c[c][:, k0:k1, :],
                            start=(i == 0),
                            stop=(i == 1),
                            perf_mode=DR,
                        )
                return S

            def emit_exp(u, S):
                """ACT: E = exp(scale*C - beta*c0), trimmed to l."""
                c = u // 4
                l = L2G[c]
                E = epool.tile([128, 2, 4, 128], BF16, tag="E", name=f"E{u}")
                Sv = S[:].rearrange("p j (g t) -> p j g t", g=4)
                nc.scalar.activation(
                    E[:, :, :, 0:l], Sv[:, :, :, 0:l], AF.Exp,
                    bias=expbias, scale=exp_scale,
                )
                return E

            def emit_sum(u, E):
                """PE: SumE row 8c+a += onehot64^T @ E.  Rows live in two
                64-row halves (legal matmul base partitions 0/64); each
                half is one PSUM accumulation chain over its 64 writes."""
                c, ap_ = u // 4, (u % 4) * 2
                l = L2G[c]
                sv = sumE[:].rearrange("p (g t) -> p g t", g=4)
                for j in range(2):
                    a = ap_ + j
                    r = 8 * c + a
                    h, m = r // 64, r % 64
                    nc.tensor.matmul(
                        sv[64 * h : 64 * (h + 1), :, 0:l],
                        oh64[:, 64 - m : 128 - m],
                        E[:, j, :, 0:l],
                        start=(m == 0),
                        stop=(m == 63),
                    )

            def emit_red(u, E):
                """sim1 maxE: optional gpsimd pre-halving, then DVE."""
                c, ap_ = u // 4, (u % 4) * 2
                l = L2G[c]
                dst = sim1st[:, ap_ : ap_ + 2, 4 * c : 4 * c + 4]
                if GPS_MOD and u % GPS_MOD == 0:
                    h = l // 2
                    H = hpool.tile([128, 2, 4, 64], BF16, tag="H", name=f"H{u}")
                    nc.gpsimd.tensor_max(
                        H[:, :, :, 0:h], E[:, :, :, 0:h], E[:, :, :, h:l]
                    )
                    nc.vector.reduce_max(dst, H[:, :, :, 0:h], axis=AX.X)
                else:
                    nc.vector.reduce_max(dst, E[:, :, :, 0:l], axis=AX.X)

            # software pipeline: PE mms for unit u, then unit u-1's
            # E-consumers (keeps the PE from stalling on ACT).
            prev = None
            for u in range(NCH * 4):
                S = emit_mm(u)
                E = emit_exp(u, S)
                if prev is not None:
                    emit_sum(prev[0], prev[1])
                    emit_red(prev[0], prev[1])
                prev = (u, E)
            emit_sum(prev[0], prev[1])
            emit_red(prev[0], prev[1])

            # ---- tails ----
            # The ACT Ln table saturates on inputs this small (~e^-70), so
            # take logs from the float exponent bits instead: int-convert
            # the raw bits; bits/2^k - 126.96 ~ log2(x) to +-0.03 ln after
            # the host applies the affine (exact for the w-means since
            # sum(w) = 1/2 per row).
            lnS = coll_pool.tile([128, 512], F32, tag="lnS")
            nc.vector.tensor_copy(lnS[:], sumE[:].bitcast(mybir.dt.uint32))
            m2t = coll_pool.tile([128, 512], F32, tag="m2t")
            nc.vector.tensor_mul(m2t[:], lnS[:], w2pat[:])
            m2col = coll_pool.tile([128, 4], F32, tag="m2col")
            nc.vector.tensor_reduce(
                m2col[:],
                m2t[:].rearrange("p (g t) -> p g t", g=4),
                axis=AX.X,
                op=ALU.add,
            )
            nc.sync.dma_start(m2o.ap(), m2col[:])

            # m1 (exact max): bit-log of maxE -> w1-weighted matmul per a.
            # full-w1T stationary writes an [8, 64] block per a at column
            # 64a; only row a of each block is wanted — host extracts the
            # diagonal band.
            ln1 = coll_pool.tile([128, A, B2], F32, tag="ln1")
            nc.vector.tensor_copy(
                ln1[:], sim1st[:].bitcast(mybir.dt.uint16)
            )
            m1ps = psM.tile([A, A * B2], F32, tag="m1ps")
            for a in range(A):
                nc.tensor.matmul(
                    m1ps[:, a * B2 : (a + 1) * B2],
                    w1T[:],
                    ln1[:, a, :],
                    start=True,
                    stop=True,
                )
            m1s = coll_pool.tile([A, A * B2], F32, tag="m1s")
            nc.scalar.copy(m1s[:], m1ps[:])
            nc.sync.dma_start(m1o.ap(), m1s[:])
    nc.finalize()
    return nc


def _prep(x1, mask1, x2, mask2):
    """Host-side marshaling: normalize, mask-zero, sort b, quantize."""
    x1 = np.asarray(x1, dtype=np.float32)
    x2 = np.asarray(x2, dtype=np.float32)
    m1 = np.asarray(mask1).astype(bool)
    m2 = np.asarray(mask2).astype(bool)

    EPS = 1e-8
    n1 = np.sqrt((x1 * x1).sum(-1, keepdims=True))
    n2 = np.sqrt((x2 * x2).sum(-1, keepdims=True))
    x1n = (x1 / np.maximum(n1, EPS)) * QSCALE
    x2n = (x2 / np.maximum(n2, EPS)) * QSCALE
    x1n[~m1] = 0.0
    x2n[~m2] = 0.0

    len1 = m1.sum(axis=1).astype(np.int64)
    len2 = m2.sum(axis=1).astype(np.int64)
    ext2 = np.where(m2.any(1), S2 - np.argmax(m2[:, ::-1], axis=1), 1)
    b_order = np.argsort(-ext2, kind="stable")

    def _ev(v):
        v = int(max(v, 2))
        return v + (v % 2)

    L2G = tuple(_ev(ext2[b_order[4 * c]]) for c in range(NCH))

    w1 = m1.astype(np.float32) * (0.5 / np.maximum(len1, 1))[:, None]
    w2 = m2.astype(np.float32) * (0.5 / np.maximum(len2, 1))[:, None]
    w2s = w2[b_order]                                 # [64 sorted b, 128 t]

    x2T = np.ascontiguousarray(x2n[b_order].reshape(X2W, D).T).astype(E4NP)
    w2pat = np.zeros((128, 512), np.float32)
    for c in range(NCH):
        for a in range(A):
            w2pat[8 * c + a] = w2s[4 * c : 4 * c + 4].reshape(512)

    in_maps = []
    for k in range(NCORES):
        rows = slice(k * A, (k + 1) * A)
        x1T = np.ascontiguousarray(
            x1n[rows].reshape(X1W, D).T
        ).astype(E4NP)
        consts = np.zeros((128, 4), np.float32)
        consts[:, 0] = 1.0
        consts[:, 1] = -BETA * C0
        consts[:, 2] = LNEPS
        w1Tc = np.ascontiguousarray(w1[rows].T)       # [128 s, 8 a]
        in_maps.append(
            {
                "x1p": x1T,
                "x2p": x2T,
                "consts": consts,
                "w2patp": w2pat,
                "w1Tp": w1Tc,
            }
        )
    return in_maps, b_order, L2G


def kernel(x1, mask1, x2, mask2):
    in_maps, b_order, key = _prep(x1, mask1, x2, mask2)
    if _CACHE.get("key") != key:
        _CACHE["nc"] = _build(key)
        _CACHE["key"] = key
    nc = _CACHE["nc"]
    res = run_bass_kernel_spmd(nc, in_maps, list(range(NCORES)))
    outp = np.zeros((B1, B2), dtype=np.float32)
    for k in range(NCORES):
        m1b = res.results[k]["m1o"].reshape(A, A, B2)  # [row, a-block, b]
        m2v = res.results[k]["m2o"]                   # [128 (c,a), 4]
        m1v = np.ascontiguousarray(
            m1b[np.arange(A), np.arange(A)]           # diagonal band
        )
        M2 = np.zeros((A, B2), np.float32)
        for c in range(NCH):
            for a in range(A):
                M2[a, 4 * c : 4 * c + 4] = m2v[8 * c + a]
        # bit-log affine: M held sum(w * bits); log2(x) ~ bits/2^k - 126.96
        LN2 = float(np.log(2.0))
        m1t = LN2 * (m1v / 128.0 - 126.9565 * 0.5)
        m2t = LN2 * (M2 / 8388608.0 - 126.9565 * 0.5)
        vals = (m1t + m2t) / BETA + C0
        outp[np.ix_(range(k * A, (k + 1) * A), b_order)] = vals
    return np.ascontiguousarray(outp)


# revision 25
# speedup vs baseline: 2.2004x; 1.2371x over previous
"""Trainium2 Bass kernel for DensePairwiseRelaxedWordMoverSimilarity.

Shapes (hardcoded): x1 (64,128,512) f32, mask1 (64,128) bool,
                    x2 (64,128,512) f32, mask2 (64,128) bool -> out (64,64) f32.

Strategy: data-parallel over B1 across 8 cores; core k holds x1 rows
[8k, 8k+8) plus the full x2 and produces an (8, 64) output slab.

v3: single-orientation fp8 matmul + exp-domain reductions.
For each (a, 4-b chunk) the PE computes C^T tiles [128 s, 4b x 128 t]
(fp8 DoubleRow, f32 PSUM).  The scalar engine applies
E = exp(beta*(C - c0)) into bf16 SBUF (trimmed to the chunk's baked
t-extent).  Then BOTH reductions come cheap:
  - sim1[a,b,s] = max_t C = (ln max_t E)/beta + c0: segmented DVE
    reduce_max over the free dim (optionally gpsimd pre-halving),
  - sim2[a,b,t] via log-sum-exp: sum_s E is a PE matmul with an
    all-ones stationary column, accumulated per (chunk, a) into one
    PSUM bank row; masked-s rows contribute exp(-beta*c0) ~ 1e-38 (b
    columns are sorted+trimmed so masked-t never pollutes sums).
    The LSE overshoot at beta=250 measures ~3e-3 rel on this data,
    under the fp8 quantization floor.
One Ln pass per collection (+1e-36 bias so memset-zero pad columns
stay finite) and tiny w1/w2-weighted matmul/reduce means finish on
device; host adds the (m1+m2)/beta + c0 affine and unsorts b.
"""

import numpy as np
import ml_dtypes

import concourse.bacc as bacc
import concourse.mybir as mybir
from concourse import tile
from concourse.bass_utils import run_bass_kernel_spmd

F32 = mybir.dt.float32
BF16 = mybir.dt.bfloat16
FP8 = mybir.dt.float8e4
AX = mybir.AxisListType
AF = mybir.ActivationFunctionType
ALU = mybir.AluOpType
DR = mybir.MatmulPerfMode.DoubleRow

E4NP = ml_dtypes.float8_e4m3
BFNP = ml_dtypes.bfloat16

B1, S1, B2, S2, D = 64, 128, 64, 128, 512
NCORES = 8
A = B1 // NCORES          # 8 x1 rows per core
X1W = A * S1              # 1024 x1 columns
X2W = B2 * S2             # 8192 x2 columns
NCH = 16                  # chunks of 4 sorted b's (512 x2 columns)
QSCALE = 32.0             # fp8 pre-scale; sims carry QSCALE^2
BETA = 250.0              # LSE sharpness
C0 = 0.35                 # exp recentering; keeps E <= ~1
LNEPS = 1e-36             # (unused since bit-log; kept in consts layout)
DEFER = 2                 # units of lag for E consumers (PE slack)

_CACHE = {}


def _build(L2G):
    """L2G: 16 baked t-extents (even), one per sorted 4-b chunk."""
    nc = bacc.Bacc(None, target_bir_lowering=False, debug=False)

    x1p = nc.declare_dram_parameter("x1p", [D, X1W], FP8, isOutput=False)
    x2p = nc.declare_dram_parameter("x2p", [D, X2W], FP8, isOutput=False)
    consts = nc.declare_dram_parameter("consts", [128, 4], F32, isOutput=False)
    w2patp = nc.declare_dram_parameter("w2patp", [128, 512], F32, isOutput=False)
    w1Tp = nc.declare_dram_parameter("w1Tp", [128, A], F32, isOutput=False)
    m1o = nc.declare_dram_parameter("m1o", [A, A * B2], F32, isOutput=True)
    m2o = nc.declare_dram_parameter("m2o", [128, 4], F32, isOutput=True)

    exp_scale = float(BETA / (QSCALE * QSCALE))

    with tile.TileContext(nc) as tc:
        with (
            tc.tile_pool(name="xts", bufs=1) as xts_pool,
            tc.tile_pool(name="cst", bufs=1) as cst_pool,
            tc.tile_pool(name="coll", bufs=1) as coll_pool,
            tc.tile_pool(name="epool", bufs=4) as epool,
            tc.tile_pool(name="hpool", bufs=3) as hpool,
            tc.tile_pool(name="psS", bufs=3, space="PSUM") as psS,
            tc.tile_pool(name="psF", bufs=1, space="PSUM") as psF,
            tc.tile_pool(name="psM", bufs=1, space="PSUM") as psM,
        ):
            # ---- loads: first chunk + x1 first (they gate the first
            # matmul), then constants, then the remaining x2 blocks.
            x2c = [None] * NCH
            blocks = [(0, 1), (1, 1), (2, 2), (4, 4), (8, 8)]
            xb0 = xts_pool.tile([128, 4, 512], FP8, tag="xb0")
            nc.sync.dma_start(
                xb0[:],
                x2p.ap()[:, 0:512].rearrange("(k p) m -> p k m", p=128),
            )
            x2c[0] = xb0[:, :, :]
            x1t = xts_pool.tile([128, 4, X1W], FP8, tag="x1t")
            nc.sync.dma_start(
                x1t[:], x1p.ap().rearrange("(k p) m -> p k m", p=128)
            )
            csts = cst_pool.tile([128, 4], F32, tag="consts")
            nc.sync.dma_start(csts[:], consts.ap())
            for g0, w in blocks[1:]:
                xb = xts_pool.tile([128, 4, w * 512], FP8, tag=f"xb{g0}")
                nc.sync.dma_start(
                    xb[:],
                    x2p.ap()[:, g0 * 512 : (g0 + w) * 512].rearrange(
                        "(k p) m -> p k m", p=128
                    ),
                )
                for j in range(w):
                    x2c[g0 + j] = xb[:, :, j * 512 : (j + 1) * 512]
            w2pat = cst_pool.tile([128, 512], F32, tag="w2pat")
            nc.sync.dma_start(w2pat[:], w2patp.ap())
            w1T = cst_pool.tile([128, A], F32, tag="w1T")
            nc.sync.dma_start(w1T[:], w1Tp.ap())

            expbias = csts[:, 1:2]      # -BETA*C0
            # sliding-window one-hot: col 64 is all-ones, so the width-64
            # slice [64-m : 128-m] has its ones in column m.
            oh64 = cst_pool.tile([128, 128], BF16, tag="oh64")
            nc.vector.memset(oh64[:], 0.0)
            nc.vector.tensor_copy(
                oh64[:, 64:65], csts[:, 0:1]
            )

            # maxE collection [s, a, sorted-b]; SumE PSUM bank rows (8c+a)
            sim1st = coll_pool.tile([128, A, B2], BF16, tag="sim1st")
            sumE = psF.tile([128, 512], F32, tag="sumE")
            nc.vector.memset(sumE[:], 0.0)

            def emit_mm(u):
                """fp8 DoubleRow matmuls for unit u = (chunk, a-pair)."""
                c, ap_ = u // 4, (u % 4) * 2
                S = psS.tile([128, 2, 512], F32, tag="S", name=f"S{u}")
                for j in range(2):
                    a = ap_ + j
                    for i, (k0, k1) in enumerate(((0, 2), (2, 4))):
                        nc.tensor.matmul(
                            S[:, j, :],
                            x1t[:, k0:k1, a * 128 : (a + 1) * 128],
                            x2c[c][:, k0:k1, :],
                            start=(i == 0),
                            stop=(i == 1),
                            perf_mode=DR,
                        )
                return S

            def emit_exp(u, S):
                """ACT: E = exp(scale*C - beta*c0), trimmed to l."""
                c = u // 4
                l = L2G[c]
                E = epool.tile([128, 2, 4, 128], BF16, tag="E", name=f"E{u}")
                Sv = S[:].rearrange("p j (g t) -> p j g t", g=4)
                nc.scalar.activation(
                    E[:, :, :, 0:l], Sv[:, :, :, 0:l], AF.Exp,
                    bias=expbias, scale=exp_scale,
                )
                return E

            def emit_sum(u, E):
                """PE: SumE row 8c+a += onehot64^T @ E.  Rows live in two
                64-row halves (legal matmul base partitions 0/64); each
                half is one PSUM accumulation chain over its 64 writes."""
                c, ap_ = u // 4, (u % 4) * 2
                l = L2G[c]
                sv = sumE[:].rearrange("p (g t) -> p g t", g=4)
                for j in range(2):
                    a = ap_ + j
                    r = 8 * c + a
                    h, m = r // 64, r % 64
                    nc.tensor.matmul(
                        sv[64 * h : 64 * (h + 1), :, 0:l],
                        oh64[:, 64 - m : 128 - m],
                        E[:, j, :, 0:l],
                        start=(m == 0),
                        stop=(m == 63),
                    )

            def emit_red(u, E):
                """sim1 maxE: one tensor_tensor max level (bf16 2x_1p)
                halves the elements the 1x-only reduce_max must stream."""
                c, ap_ = u // 4, (u % 4) * 2
                l = L2G[c]
                dst = sim1st[:, ap_ : ap_ + 2, 4 * c : 4 * c + 4]
                h = l // 2
                H = hpool.tile([128, 2, 4, 64], BF16, tag="H", name=f"H{u}")
                nc.vector.tensor_max(
                    H[:, :, :, 0:h], E[:, :, :, 0:h], E[:, :, :, h:l]
                )
                nc.vector.reduce_max(dst, H[:, :, :, 0:h], axis=AX.X)

            # software pipeline: PE mms for unit u, then unit u-DEFER's
            # E-consumers (keeps the PE from stalling on ACT).
            pending = []
            for u in range(NCH * 4):
                S = emit_mm(u)
                E = emit_exp(u, S)
                pending.append((u, E))
                if len(pending) > DEFER:
                    pu, pE = pending.pop(0)
                    emit_sum(pu, pE)
                    emit_red(pu, pE)
            for pu, pE in pending:
                emit_sum(pu, pE)
                emit_red(pu, pE)

            # ---- tails ----
            # The ACT Ln table saturates on inputs this small (~e^-70), so
            # take logs from the float exponent bits instead: int-convert
            # the raw bits; bits/2^k - 126.96 ~ log2(x) to +-0.03 ln after
            # the host applies the affine (exact for the w-means since
            # sum(w) = 1/2 per row).
            lnS = coll_pool.tile([128, 512], F32, tag="lnS")
            nc.vector.tensor_copy(lnS[:], sumE[:].bitcast(mybir.dt.uint32))
            m2t = coll_pool.tile([128, 512], F32, tag="m2t")
            nc.vector.tensor_mul(m2t[:], lnS[:], w2pat[:])
            m2col = coll_pool.tile([128, 4], F32, tag="m2col")
            nc.vector.tensor_reduce(
                m2col[:],
                m2t[:].rearrange("p (g t) -> p g t", g=4),
                axis=AX.X,
                op=ALU.add,
            )
            nc.sync.dma_start(m2o.ap(), m2col[:])

            # m1 (exact max): bit-log of maxE -> w1-weighted matmul per a.
            # full-w1T stationary writes an [8, 64] block per a at column
            # 64a; only row a of each block is wanted — host extracts the
            # diagonal band.
            ln1 = coll_pool.tile([128, A, B2], F32, tag="ln1")
            nc.vector.tensor_copy(
                ln1[:], sim1st[:].bitcast(mybir.dt.uint16)
            )
            m1ps = psM.tile([A, A * B2], F32, tag="m1ps")
            for a in range(A):
                nc.tensor.matmul(
                    m1ps[:, a * B2 : (a + 1) * B2],
                    w1T[:],
                    ln1[:, a, :],
                    start=True,
                    stop=True,
                )
            m1s = coll_pool.tile([A, A * B2], F32, tag="m1s")
            nc.scalar.copy(m1s[:], m1ps[:])
            nc.sync.dma_start(m1o.ap(), m1s[:])
    nc.finalize()
    return nc


def _prep(x1, mask1, x2, mask2):
    """Host-side marshaling: normalize, mask-zero, sort b, quantize."""
    x1 = np.asarray(x1, dtype=np.float32)
    x2 = np.asarray(x2, dtype=np.float32)
    m1 = np.asarray(mask1).astype(bool)
    m2 = np.asarray(mask2).astype(bool)

    EPS = 1e-8
    n1 = np.sqrt((x1 * x1).sum(-1, keepdims=True))
    n2 = np.sqrt((x2 * x2).sum(-1, keepdims=True))
    x1n = (x1 / np.maximum(n1, EPS)) * QSCALE
    x2n = (x2 / np.maximum(n2, EPS)) * QSCALE
    x1n[~m1] = 0.0
    x2n[~m2] = 0.0

    len1 = m1.sum(axis=1).astype(np.int64)
    len2 = m2.sum(axis=1).astype(np.int64)
    ext2 = np.where(m2.any(1), S2 - np.argmax(m2[:, ::-1], axis=1), 1)
    b_order = np.argsort(-ext2, kind="stable")

    def _ev(v):
        v = int(max(v, 4))
        return (v + 3) // 4 * 4   # /2-able and 4B-aligned halves (2x_1p)

    L2G = tuple(_ev(ext2[b_order[4 * c]]) for c in range(NCH))

    w1 = m1.astype(np.float32) * (0.5 / np.maximum(len1, 1))[:, None]
    w2 = m2.astype(np.float32) * (0.5 / np.maximum(len2, 1))[:, None]
    w2s = w2[b_order]                                 # [64 sorted b, 128 t]

    x2T = np.ascontiguousarray(x2n[b_order].reshape(X2W, D).T).astype(E4NP)
    w2pat = np.zeros((128, 512), np.float32)
    for c in range(NCH):
        for a in range(A):
            w2pat[8 * c + a] = w2s[4 * c : 4 * c + 4].reshape(512)

    in_maps = []
    for k in range(NCORES):
        rows = slice(k * A, (k + 1) * A)
        x1T = np.ascontiguousarray(
            x1n[rows].reshape(X1W, D).T
        ).astype(E4NP)
        consts = np.zeros((128, 4), np.float32)
        consts[:, 0] = 1.0
        consts[:, 1] = -BETA * C0
        consts[:, 2] = LNEPS
        w1Tc = np.ascontiguousarray(w1[rows].T)       # [128 s, 8 a]
        in_maps.append(
            {
                "x1p": x1T,
                "x2p": x2T,
                "consts": consts,
                "w2patp": w2pat,
                "w1Tp": w1Tc,
            }
        )
    return in_maps, b_order, L2G


def kernel(x1, mask1, x2, mask2):
    in_maps, b_order, key = _prep(x1, mask1, x2, mask2)
    if _CACHE.get("key") != key:
        _CACHE["nc"] = _build(key)
        _CACHE["key"] = key
    nc = _CACHE["nc"]
    res = run_bass_kernel_spmd(nc, in_maps, list(range(NCORES)))
    outp = np.zeros((B1, B2), dtype=np.float32)
    for k in range(NCORES):
        m1b = res.results[k]["m1o"].reshape(A, A, B2)  # [row, a-block, b]
        m2v = res.results[k]["m2o"]                   # [128 (c,a), 4]
        m1v = np.ascontiguousarray(
            m1b[np.arange(A), np.arange(A)]           # diagonal band
        )
        M2 = np.zeros((A, B2), np.float32)
        for c in range(NCH):
            for a in range(A):
                M2[a, 4 * c : 4 * c + 4] = m2v[8 * c + a]
        # bit-log affine: M held sum(w * bits); log2(x) ~ bits/2^k - 126.96
        LN2 = float(np.log(2.0))
        m1t = LN2 * (m1v / 128.0 - 126.9565 * 0.5)
        m2t = LN2 * (M2 / 8388608.0 - 126.9565 * 0.5)
        vals = (m1t + m2t) / BETA + C0
        outp[np.ix_(range(k * A, (k + 1) * A), b_order)] = vals
    return np.ascontiguousarray(outp)
